# revision 19
# baseline (speedup 1.0000x reference)
"""Multi-head self-attention (full-embed, no head split) on 8 Trainium2 cores.

Sharding: data-parallel over (batch=4) x (query-half=2) = 8 cores.
Each core computes V for the full 2048-row sequence of its batch (duplicated
across the core pair), attention scores for its 1024 query rows, softmax,
weights @ V, and the output projection for its rows.

All device matmuls run as float32r (TF32-like, full PE rate); accumulation is
fp32. Key algebraic folds (all host-side, weight-only precomputes):
  - Q and K only enter via Q.K^T, so the two projections are fused into one
    matrix M = Wk^T @ Wq (host, fp64). On device: D = M-matmul of x^T, then
    scores^T = X @ D. This removes the Q and K projections entirely.
  - Score bias terms: the q-dependent parts cancel in softmax; the
    k-dependent part is sum_e X[k,e]*a2[e] with a2 = Wk^T @ bq, which folds
    into D's per-partition bias during the PSUM->SBUF copy.
  - V and Wo only appear as (softmax.V).Wo^T = softmax.(X @ (Wo@Wv)^T), so
    they are fused into one host matrix G = Wo @ Wv; the device projects
    U = X @ G^T once and the AV matmul directly yields the final output
    (normalize + bias fused into its PSUM drain). This removes the output
    projection entirely. The V bias folds into the output bias
    (bo' = bo + Wo @ bv, exact since softmax weights sum to 1).
Layout choices:
  - x is passed transposed per core as xt [E, S], with the core's query half
    permuted to the front (softmax over k is permutation-invariant as long as
    V uses the same k order, which it does); xt stays resident and serves as
    the stationary operand for both V-projection and the scores matmul.
  - scores^T is [k, q] so AV needs no transpose of the softmax weights; the
    softmax denominator Z is a [1, q] row via a ones-column matmul, broadcast
    to [128, q] via a K=1 ones-row matmul.
  - U (natural [s, f] layout): the f<512 half stays resident in SBUF; only
    the f>=512 half is spilled to DRAM and streamed back during AV in
    [128, 512] chunks.
  - The output projection is computed transposed (out^T [f, q]) so its weight
    tiles stream as small slices and its bias is per-partition; the host
    transposes the result back.
"""
import sys

sys.path.insert(0, '/opt/trn_rl_repo')

import numpy as np

import concourse.bass as bass
import concourse.bacc as bacc
import concourse.tile as tile
import concourse.mybir as mybir
from concourse import bass_utils

F32 = mybir.dt.float32
F32R = mybir.dt.float32r
AF = mybir.ActivationFunctionType

N_CORES = 8
B, S, E = 4, 2048, 1024
SH = S // 2          # per-core query rows
P = 128
EO = E // P          # 8 contraction chunks
FO = E // P          # 8 output-feature chunks
KO = S // P          # 16 key chunks
QB = 512             # q block (PSUM free dim)
NQB = SH // QB       # 2 q blocks per core
SCALE = 1.0 / np.sqrt(np.float32(E))

_CACHE = {}


def build_nc(loop_iters=None, stagger=True, vch_bufs=6):
    """Build + compile the Bass module. loop_iters wraps the whole body in a
    hardware loop (used only for timing amplification by test harnesses)."""
    nc = bacc.Bacc("TRN2", target_bir_lowering=False, debug=False,
                   enable_asserts=False, num_devices=N_CORES)

    xt_ap = nc.dram_tensor("xt", [E, S], F32R, kind="ExternalInput").ap()
    mt_ap = nc.dram_tensor("mt", [EO, P, EO, P], F32R, kind="ExternalInput").ap()
    wv_ap = nc.dram_tensor("wv", [2, P, EO, 512], F32R, kind="ExternalInput").ap()
    a2r_ap = nc.dram_tensor("a2r", [P, EO], F32, kind="ExternalInput").ap()
    bor_ap = nc.dram_tensor("bor", [P, FO], F32, kind="ExternalInput").ap()
    ones_ap = nc.dram_tensor("ones", [P, P], F32R, kind="ExternalInput").ap()
    # transposed output; host transposes back
    out_ap = nc.dram_tensor("out", [E, SH], F32, kind="ExternalOutput").ap()

    with tile.TileContext(nc) as tc:
        persist = tc.alloc_tile_pool(name="persist", bufs=1)
        dramp = tc.alloc_tile_pool(name="dramp", bufs=1, space="DRAM")

        def body():
            xt_sb = persist.tile([P, EO, S], F32R, name="xt_sb")
            ones_sb = persist.tile([P, P], F32R, name="ones_sb")
            u0_sb = persist.tile([P, KO, 512], F32R, name="u0_sb")
            v_dram = dramp.tile([KO, P, 512], F32R, name="v_dram")

            psB = tc.alloc_tile_pool(name="psB", bufs=1, space="PSUM")

            # blk_b is allocated before vpool (they coexist) so block-0's
            # score-weight slices can be queued at the head of the DMA queue
            # and D(0) never waits on the V-phase DMA tail.
            blk_b = tc.alloc_tile_pool(name="blk_b", bufs=1)

            # ---- Phase 1: V (natural [s, f], bias-free) -> DRAM spill.
            # wv half 0 is queued before the 8MB xt transfer so the first
            # matmuls only wait on xt chunk 0.
            vpool = tc.alloc_tile_pool(name="vpool", bufs=1)
            wv_next = vpool.tile([P, EO, 512], F32R, tag="wv", bufs=2,
                                 name="wv_t")
            nc.sync.dma_start(out=wv_next, in_=wv_ap[0])
            mt_first = blk_b.tile([P, EO, P], F32R, tag="mt", bufs=2,
                                  name="mt_t")
            nc.sync.dma_start(out=mt_first, in_=mt_ap[0])
            a2r_sb = blk_b.tile([P, EO], F32, tag="a2r", bufs=1, name="a2r_sb")
            nc.sync.dma_start(out=a2r_sb, in_=a2r_ap)
            bor_sb = blk_b.tile([P, FO], F32, tag="bor", bufs=1, name="bor_sb")
            nc.sync.dma_start(out=bor_sb, in_=bor_ap)
            for eo in range(EO):
                nc.sync.dma_start(out=xt_sb[:, eo, :],
                                  in_=xt_ap[eo * P:(eo + 1) * P, :])
            nc.sync.dma_start(out=ones_sb, in_=ones_ap)

            for ft in range(2):
                wv_t = wv_next
                if ft + 1 < 2:
                    wv_next = vpool.tile([P, EO, 512], F32R, tag="wv", bufs=2,
                                         name="wv_t")
                    nc.sync.dma_start(out=wv_next, in_=wv_ap[ft + 1])
                for so in range(KO):
                    psv = psB.tile([P, 512], F32, tag="psv", bufs=2, name="psv")
                    for eo in range(EO):
                        nc.tensor.matmul(psv, lhsT=xt_sb[:, eo, so * P:(so + 1) * P],
                                         rhs=wv_t[:, eo, :],
                                         start=(eo == 0), stop=(eo == EO - 1))
                    if ft == 0:
                        # f<512 half of U stays resident in SBUF
                        with nc.allow_low_precision(
                                reason="U feeds fp32r AU matmul"):
                            nc.vector.tensor_copy(out=u0_sb[:, so, :], in_=psv)
                    else:
                        vst = vpool.tile([P, 512], F32R, tag="vst", bufs=2,
                                         name="vst")
                        with nc.allow_low_precision(
                                reason="U feeds fp32r AU matmul"):
                            nc.vector.tensor_copy(out=vst, in_=psv)
                        nc.sync.dma_start(out=v_dram[so], in_=vst)
            vpool.release()
            psB.release()

            # ---- Phase 2: per q-block attention + output projection.
            # ps_mid is allocated first so it (not ps_sc) lands on the V-phase
            # psum banks: its first use (Z) is late, while ps_sc (D/scores)
            # starts immediately after the V matmuls.
            ps_mid = tc.alloc_tile_pool(name="ps_mid", bufs=1, space="PSUM")
            ps_sc = tc.alloc_tile_pool(name="ps_sc", bufs=1, space="PSUM")
            blk = tc.alloc_tile_pool(name="blk", bufs=1)

            def emit_scores(qb, mt0=None):
                """D = M . x^T (+ a2 bias) -> scores^T -> exp -> Z -> zinv
                -> broadcast."""
                q0 = qb * QB
                exp_sb = blk.tile([P, KO, QB], F32R, tag="exp", bufs=1,
                                  name="exp_sb")
                d_sb = blk.tile([P, EO, QB], F32R, tag="d", bufs=1, name="d_sb")
                if mt0 is not None:
                    mt_next = mt0
                else:
                    mt_next = blk_b.tile([P, EO, P], F32R, tag="mt", bufs=2,
                                         name="mt_t")
                    nc.sync.dma_start(out=mt_next, in_=mt_ap[0])
                for eod in range(EO):
                    mt_t = mt_next
                    if eod + 1 < EO:
                        mt_next = blk_b.tile([P, EO, P], F32R, tag="mt", bufs=2,
                                             name="mt_t")
                        nc.sync.dma_start(out=mt_next, in_=mt_ap[eod + 1])
                    psd = ps_sc.tile([P, QB], F32, tag="pss", bufs=2, name="psd")
                    for eo in range(EO):
                        nc.tensor.matmul(psd, lhsT=mt_t[:, eo, :],
                                         rhs=xt_sb[:, eo, q0:q0 + QB],
                                         start=(eo == 0), stop=(eo == EO - 1))
                    nc.scalar.activation(out=d_sb[:, eod, :], in_=psd,
                                         func=AF.Identity,
                                         bias=a2r_sb[:, eod:eod + 1], scale=1.0)

                for ko in range(KO):
                    pss = ps_sc.tile([P, QB], F32, tag="pss", bufs=2, name="pss")
                    for eo in range(EO):
                        nc.tensor.matmul(pss, lhsT=xt_sb[:, eo, ko * P:(ko + 1) * P],
                                         rhs=d_sb[:, eo, :],
                                         start=(eo == 0), stop=(eo == EO - 1))
                    nc.scalar.activation(out=exp_sb[:, ko, :], in_=pss,
                                         func=AF.Exp, scale=float(SCALE))
                psz = ps_mid.tile([P, QB], F32, tag="psa", bufs=4, name="psz")
                for ko in range(KO):
                    nc.tensor.matmul(psz[:1, :], lhsT=ones_sb[:, 0:1],
                                     rhs=exp_sb[:, ko, :],
                                     start=(ko == 0), stop=(ko == KO - 1))
                zinv = blk.tile([1, QB], F32R, tag="zinv", bufs=1, name="zinv")
                with nc.allow_low_precision(reason="zinv feeds fp32r matmul"):
                    nc.vector.reciprocal(out=zinv[:1, :], in_=psz[:1, :])
                psb = ps_sc.tile([P, QB], F32, tag="pss", bufs=2, name="psb")
                nc.tensor.matmul(psb, lhsT=ones_sb[:1, :], rhs=zinv[:1, :],
                                 start=True, stop=True)
                zb_sb = blk.tile([P, QB], F32, tag="zb", bufs=1, name="zb_sb")
                nc.vector.tensor_copy(out=zb_sb, in_=psb)
                return exp_sb, zb_sb

            def emit_au_out(qb, exp_sb, zb_sb):
                """out^T[f, q] = (sum_k U[k, f] * exp[k, q]) * zinv[q] + bo'"""
                q0 = qb * QB
                for ft in range(2):
                    psp = [ps_mid.tile([P, QB], F32, tag="psa", bufs=4,
                                       name=f"psa{j}") for j in range(4)]
                    for ko in range(KO):
                        if ft == 0:
                            usrc = u0_sb[:, ko, :]
                        else:
                            vch = blk.tile([P, 512], F32R, tag="vch",
                                           bufs=vch_bufs, name="vch")
                            nc.sync.dma_start(out=vch, in_=v_dram[ko])
                            usrc = vch
                        for j in range(4):
                            nc.tensor.matmul(psp[j],
                                             lhsT=usrc[:, j * P:(j + 1) * P],
                                             rhs=exp_sb[:, ko, :],
                                             start=(ko == 0),
                                             stop=(ko == KO - 1))
                    for j in range(4):
                        fo = ft * 4 + j
                        osa = blk_b.tile([P, QB], F32, tag="osa", bufs=2,
                                         name="osa")
                        nc.vector.tensor_mul(out=osa, in0=psp[j], in1=zb_sb)
                        ost = blk_b.tile([P, QB], F32, tag="ost", bufs=2,
                                         name="ost")
                        nc.scalar.activation(out=ost, in_=osa, func=AF.Identity,
                                             bias=bor_sb[:, fo:fo + 1], scale=1.0)
                        nc.sync.dma_start(
                            out=out_ap[fo * P:(fo + 1) * P, q0:q0 + QB], in_=ost)

            for qb in range(NQB):
                expq, zbq = emit_scores(qb, mt0=mt_first if qb == 0 else None)
                emit_au_out(qb, expq, zbq)

            blk.release()
            blk_b.release()
            ps_sc.release()
            ps_mid.release()

        if loop_iters is None:
            body()
        else:
            with tc.For_i(0, loop_iters):
                body()

        dramp.release()
        persist.release()

    nc.compile()
    return nc


def _prep_shared(Wq, bq, Wk, bk, Wv, bv, Wo, bo):
    def chunk_w(W, free):
        wT = np.ascontiguousarray(np.asarray(W, dtype=np.float32).T)
        n = E // free
        return np.ascontiguousarray(
            wT.reshape(EO, P, n, free).transpose(2, 1, 0, 3))

    W64 = {k: np.asarray(v, dtype=np.float64)
           for k, v in dict(Wq=Wq, bq=bq, Wk=Wk, Wv=Wv, bv=bv, Wo=Wo,
                            bo=bo).items()}
    # Q.K^T and V.Wo^T weight fusions (see module docstring)
    M = (W64["Wk"].T @ W64["Wq"]).astype(np.float32)      # [e, e']
    G = (W64["Wo"] @ W64["Wv"]).astype(np.float32)        # [f, e']
    a2 = (W64["Wk"].T @ W64["bq"]).astype(np.float32)     # [e]
    bo_folded = (W64["bo"] + W64["Wo"] @ W64["bv"]).astype(np.float32)
    return {
        "mt": chunk_w(M, P),
        "wv": chunk_w(G, 512),
        "a2r": np.ascontiguousarray(a2.reshape(EO, P).T),
        "bor": np.ascontiguousarray(bo_folded.reshape(FO, P).T),
        "ones": np.ones((P, P), dtype=np.float32),
    }


def make_in_maps(x, Wq, bq, Wk, bk, Wv, bv, Wo, bo):
    shared = _prep_shared(Wq, bq, Wk, bk, Wv, bv, Wo, bo)
    in_maps = []
    for c in range(N_CORES):
        b, h = c // 2, c % 2
        xt = np.asarray(x[b]).T  # [E, S]
        if h == 0:
            xt_p = np.ascontiguousarray(xt)
        else:
            xt_p = np.ascontiguousarray(
                np.concatenate([xt[:, SH:], xt[:, :SH]], axis=1))
        m = {"xt": xt_p}
        m.update(shared)
        in_maps.append(m)
    return in_maps


def kernel(x, Wq, bq, Wk, bk, Wv, bv, Wo, bo):
    x = np.asarray(x, dtype=np.float32)
    args = [np.asarray(a, dtype=np.float32)
            for a in (Wq, bq, Wk, bk, Wv, bv, Wo, bo)]
    if "nc" not in _CACHE:
        _CACHE["nc"] = build_nc()
    nc = _CACHE["nc"]
    in_maps = make_in_maps(x, *args)
    res = bass_utils.run_bass_kernel_spmd(nc, in_maps,
                                          core_ids=list(range(N_CORES)))
    out = np.empty((B, S, E), dtype=np.float32)
    for c in range(N_CORES):
        b, h = c // 2, c % 2
        out[b, h * SH:(h + 1) * SH, :] = res.results[c]["out"].T
    return out
